# revision 24
# baseline (speedup 1.0000x reference)
"""MultiHeadAttention (QKV proj + softmax attention + residual + LayerNorm)
for Trainium2, SPMD across 8 NeuronCores.

Sharding: data-parallel over (batch, query-L-half): core c handles batch c//2,
query rows [1024*(c%2), 1024*(c%2)+1024), all 12 heads, full 2048 keys.
No cross-core communication.

Structure (v10, ~371us vs 606us staged baseline):
- Paired-chunk input DMAs on the two HWDGE rings (sync/scalar), ordered by
  first use; gpsimd SWDGE only carries late-needed tensors.
- Q/K projection e-chunk 0 runs first; attention follows with the V
  projection and remaining Q/K projection chunks emitted as PE filler
  inside the attention loop.
- The attention inner loop is paced by the score-psum ping-pong cycle
  (scores -> exp -> scores, single-buffered due to the 8-bank PSUM budget).
  The two heads of a pair therefore run their exp on DIFFERENT engines
  (head0 exact exp on ScalarE; head1 Schraudolph bit-trick on VectorE:
  int8(A*s+B) bitcast to fp8e4m3), so the two cycles progress in parallel;
  attnV trails scores by one kp so nothing waits on a just-issued exp.
- attnV runs in fp8-e4m3 DoubleRow (contraction 256 = both k-chunks of a
  kp in one matmul). exp outputs are fp8 with a -4 shift (softmax is
  shift-invariant; keeps e^s inside e4m3 range, max score ~8.5).
- Out-path copies/normalize run on ScalarE (attn outputs are >=0, so Relu
  with per-partition scale=1/denominator is an exact normalize), keeping
  VectorE free for its exp share. Residual-add + bn_stats run inline per
  128-column chunk; the layernorm tail only aggregates + scales. Final
  output bf16 (restored to f32 on host).

Numerics: projections/scores in bf16 (fp32 accumulate), attnV fp8,
normalization + layernorm f32, residual/gamma/beta/output bf16. Scale
A16/8 (A16=2^7/ln2) is folded into Wq/bq on the host. End-to-end rel err
~6e-3 vs the 2e-2 gate (validated in numpy sim + on HW).
"""

import sys

sys.path.insert(0, "/opt/trn_rl_repo")

import numpy as np
import ml_dtypes

N_CORES = 8
B, L, D = 4, 2048, 768
H, DK = 12, 64
LQ = L // 2  # 1024 query rows per core
LK = L  # full keys per core
DT = D // 128  # 6 d-chunks
NKC = LK // 128  # 16 k-chunks
NKP = NKC // 2  # 8 kp iterations (2 k-chunks each)
VH = 80  # per-head stride in the fp8 V tile (16B-aligned)

A16 = 128.0 / float(np.log(2.0))  # folded score scale (2^7/ln2)
CSH = 4.0  # softmax shift: exp(s - CSH)
A8 = 8.0 / float(np.log(2.0))  # e4m3 Schraudolph scale
B8 = 56.0 - A8 * CSH - 0.47  # e4m3 exponent bias - shift - mid correction


def _use_dve_exp(hp, qb, kp, p):
    """Head p1's exp runs on VectorE (Schraudolph) so the two score-psum
    ping-pong cycles pace on different engines in parallel."""
    return p == 1


_COMPILED = None


def _emit(tc, aps):
    import contextlib

    import concourse.bass as bass
    from concourse import mybir

    nc = tc.nc
    f32 = mybir.dt.float32
    bf16 = mybir.dt.bfloat16
    fp8 = mybir.dt.float8e4
    i8 = mybir.dt.int8
    AF = mybir.ActivationFunctionType
    ALU = mybir.AluOpType
    PM = mybir.MatmulPerfMode

    qT, kT, qres, wqT, wkT, wvT, bq8, bkv, bvb, gam, bet, iden, out = aps

    ctx = contextlib.ExitStack()
    with ctx:
        const = ctx.enter_context(tc.tile_pool(name="const", bufs=1))
        persist = ctx.enter_context(tc.tile_pool(name="persist", bufs=1))
        # PSUM (8 banks of 2KB): s0,s1 [128,1024]f32 = 2 banks each; po0,po1
        # [65,512] 1 bank each (transpose tiles reuse them); pj0,pj1
        # (projection accumulators, ping-pong) 1 bank each.
        ps_score = ctx.enter_context(tc.tile_pool(name="ps_score", bufs=1, space="PSUM"))
        ps_attn = ctx.enter_context(tc.tile_pool(name="ps_attn", bufs=1, space="PSUM"))
        ps_proj = ctx.enter_context(tc.tile_pool(name="ps_proj", bufs=1, space="PSUM"))
        expp = ctx.enter_context(tc.tile_pool(name="expp", bufs=6))
        osb = ctx.enter_context(tc.tile_pool(name="osb", bufs=2))
        smalls = ctx.enter_context(tc.tile_pool(name="smalls", bufs=4))
        qrp = ctx.enter_context(tc.tile_pool(name="qrp", bufs=1))
        statp = ctx.enter_context(tc.tile_pool(name="statp", bufs=4))
        fop = ctx.enter_context(tc.tile_pool(name="fop", bufs=2))

        # ---------------- input tiles ----------------
        # Paired-chunk dma_starts (amortize per-transfer fixed cost) spread
        # round-robin over four issue queues; order = first use.
        _rings = [nc.sync, nc.scalar]
        _ring_rr = [0]

        def _dma(out_ap, in_ap):
            eng = _rings[_ring_rr[0] % len(_rings)]
            _ring_rr[0] += 1
            eng.dma_start(out=out_ap, in_=in_ap)

        def big_tile(name, ncols):
            t = const.tile([128, DT, ncols], bf16, tag=name, name=name)
            return t, [t[:, i, :] for i in range(DT)]

        wq_t, wq_sb = big_tile("wqc", D)
        qT_t, qT_sb = big_tile("qTc", LQ)
        wk_t, wk_sb = big_tile("wkc", D)
        kT_t, kT_sb = big_tile("kTc", LK)
        wv_t, wv_sb = big_tile("wvc", D)

        def pair_dmas(t, dram, ncols, c0=0, c1=None):
            for i in range(DT // 2):
                src = bass.AP(
                    tensor=dram.tensor,
                    offset=dram.offset + 256 * i * ncols + c0,
                    ap=[[ncols, 128], [128 * ncols, 2], [1, (c1 or ncols) - c0]],
                )
                _dma(t[:, 2 * i : 2 * i + 2, c0 : c1 or ncols], src)

        pair_dmas(wq_t, wqT, D)
        pair_dmas(qT_t, qT, LQ)
        pair_dmas(wk_t, wkT, D)
        pair_dmas(kT_t, kT, LK, 0, 1024)
        pair_dmas(wv_t, wvT, D)
        pair_dmas(kT_t, kT, LK, 1024, 2048)

        def load_bias(name, dram, eng):
            t = const.tile([128, DT, 1], f32, tag=name, name=name)
            src = bass.AP(
                tensor=dram.tensor, offset=dram.offset, ap=[[1, 128], [128, DT], [0, 1]]
            )
            eng.dma_start(out=t, in_=src)
            return [t[:, i, :] for i in range(DT)]

        bq_sb = load_bias("bq", bq8, nc.sync)
        bk_sb = load_bias("bk", bkv, nc.scalar)
        bv_sb = const.tile([1, D], bf16, tag="bv", name="bv_sb")
        nc.sync.dma_start(out=bv_sb, in_=bvb[:])
        id_sb = const.tile([65, 65], f32, tag="iden", name="id_sb")
        nc.gpsimd.dma_start(out=id_sb, in_=iden[0:65, 0:65])
        gam_sb = const.tile([128, D], bf16, tag="gam", name="gam_sb")
        nc.gpsimd.dma_start(
            out=gam_sb,
            in_=bass.AP(tensor=gam.tensor, offset=gam.offset, ap=[[0, 128]] + list(gam.ap)),
        )
        bet_sb = const.tile([128, D], bf16, tag="bet", name="bet_sb")
        nc.gpsimd.dma_start(
            out=bet_sb,
            in_=bass.AP(tensor=bet.tensor, offset=bet.offset, ap=[[0, 128]] + list(bet.ap)),
        )
        ones_sb = const.tile([1, 128], bf16, tag="ones", name="ones_sb")
        nc.vector.memset(ones_sb, 1.0)
        eps_sb = const.tile([128, 1], f32, tag="eps", name="eps_sb")
        nc.vector.memset(eps_sb, 1e-5)
        csh_sb = const.tile([128, 1], f32, tag="csh", name="csh_sb")
        nc.vector.memset(csh_sb, -CSH)

        # residual tiles (first needed ~45us in)
        qr_sb = [qrp.tile([128, D], f32, tag=f"qr{qi}", name=f"qr{qi}") for qi in range(LQ // 128)]
        for qi in range(LQ // 128):
            (nc.gpsimd if qi % 2 else nc.sync).dma_start(
                out=qr_sb[qi], in_=qres[128 * qi : 128 * (qi + 1), :]
            )

        # persistent intermediates
        pq_sb = [persist.tile([128, LQ], bf16, tag=f"pq{i}", name=f"pq_sb{i}") for i in range(DT)]
        pk_sb = [persist.tile([128, LK], bf16, tag=f"pk{i}", name=f"pk_sb{i}") for i in range(DT)]
        # V in fp8, kc pairs interleaved for DoubleRow: [l-part, pair, head, VH]
        v_sb = [
            persist.tile([128, 2, H, VH], fp8, tag=f"v{t}", name=f"v_sb{t}")
            for t in range(NKP)
        ]
        om_sb = [persist.tile([128, D], f32, tag=f"om{q}", name=f"om_sb{q}") for q in range(LQ // 128)]
        st_sb = [persist.tile([128, DT, 6], f32, tag=f"st{q}", name=f"st_sb{q}") for q in range(LQ // 128)]

        # ---------------- projection units (emitted as PE filler) ----------
        _pj_rr = [0]

        def _pj_tile():
            ps = ps_proj.tile([128, 512], f32, tag=f"pj{_pj_rr[0] % 2}", name="ps_p")
            _pj_rr[0] += 1
            return ps

        def proj_qk_unit(i, which, lb):
            src_tiles, w_tiles, b_tiles, dst = (
                (qT_sb, wq_sb, bq_sb, pq_sb) if which == "q" else (kT_sb, wk_sb, bk_sb, pk_sb)
            )
            ps = _pj_tile()
            for kc in range(DT):
                nc.tensor.matmul(
                    ps,
                    lhsT=w_tiles[kc][:, 128 * i : 128 * (i + 1)],
                    rhs=src_tiles[kc][:, 512 * lb : 512 * (lb + 1)],
                    start=kc == 0,
                    stop=kc == DT - 1,
                )
            nc.scalar.activation(
                out=dst[i][:, 512 * lb : 512 * (lb + 1)],
                in_=ps,
                func=AF.Relu,
                bias=b_tiles[i],
            )

        QK_UNITS = [("q", 0), ("q", 1), ("k", 0), ("k", 1), ("k", 2), ("k", 3)]

        def proj_v_unit(t):
            kp2, sl = divmod(t, 2)
            if sl == 0:
                nc.vector.memset(v_sb[kp2][:, :, :, DK : DK + 1], 1.0)
            for ei, (e0, ew) in enumerate(((0, 512), (512, 256))):
                ps = _pj_tile()
                for kc in range(DT):
                    nc.tensor.matmul(
                        ps[:, 0:ew],
                        lhsT=kT_sb[kc][:, 128 * t : 128 * (t + 1)],
                        rhs=wv_sb[kc][:, e0 : e0 + ew],
                        start=kc == 0,
                        stop=False,
                    )
                nc.tensor.matmul(
                    ps[:, 0:ew],
                    lhsT=ones_sb[:, 0:128],
                    rhs=bv_sb[:, e0 : e0 + ew],
                    start=False,
                    stop=True,
                )
                nc.scalar.activation(
                    out=v_sb[kp2][:, sl, e0 // DK : (e0 + ew) // DK, 0:DK],
                    in_=ps[:, 0:ew].rearrange("p (h x) -> p h x", x=DK),
                    func=AF.Relu,
                )

        # ---------------- attention ----------------
        def attention(hp, qb, filler):
            po = [
                ps_attn.tile([DK + 1, 512], f32, tag=f"po{p}", name=f"ps_o{p}")
                for p in range(2)
            ]

            def attn_v1(kp, p, e_t):
                # one fp8 DoubleRow matmul: contraction over both k-chunks
                # of kp (pair dim), accumulating into po[p].
                h = 2 * hp + p
                nc.tensor.matmul(
                    po[p],
                    lhsT=v_sb[kp][:, :, h, 0 : DK + 1],
                    rhs=e_t.rearrange("x (two q) -> x two q", two=2),
                    start=kp == 0,
                    stop=kp == NKP - 1,
                    perf_mode=PM.DoubleRow,
                )

            def scores(kp, p, ps):
                # scores^T [k-part, q-free]; head 2hp+p row-tiled (K=64)
                for half in range(2):
                    kc = 2 * kp + half
                    nc.tensor.matmul(
                        ps[:, 512 * half : 512 * (half + 1)],
                        lhsT=pk_sb[hp][64 * p : 64 * (p + 1), 128 * kc : 128 * (kc + 1)],
                        rhs=pq_sb[hp][64 * p : 64 * (p + 1), 512 * qb : 512 * (qb + 1)],
                        start=True,
                        stop=True,
                        tile_position=(64 * p, 0),
                    )

            def exp(kp, p, ps):
                e_t = expp.tile([128, 1024], fp8, tag="exp", bufs=8, name="e_t")
                if _use_dve_exp(hp, qb, kp, p):
                    nc.vector.tensor_scalar(
                        out=e_t.bitcast(i8),
                        in0=ps,
                        scalar1=1.0 / 16.0,
                        scalar2=B8,
                        op0=ALU.mult,
                        op1=ALU.add,
                    )
                else:
                    nc.scalar.activation(
                        out=e_t, in_=ps, func=AF.Exp, scale=1.0 / A16, bias=csh_sb
                    )
                return e_t

            # Software pipeline: each head's score matmuls sit ~one block
            # after the exp that frees their psum tile, so nothing stalls;
            # attnV for kp-1 consumes the previous block's exp output.
            prev_e = [None, None]
            for kp in range(NKP):
                pspair = [
                    ps_score.tile([128, 1024], f32, tag=f"s{p}", name=f"ps_s{p}")
                    for p in range(2)
                ]
                scores(kp, 0, pspair[0])
                e0 = exp(kp, 0, pspair[0])
                if kp > 0:
                    attn_v1(kp - 1, 0, prev_e[0])
                scores(kp, 1, pspair[1])
                e1 = exp(kp, 1, pspair[1])
                if kp > 0:
                    attn_v1(kp - 1, 1, prev_e[1])
                for f in filler[kp]:
                    f()
                prev_e = [e0, e1]
            attn_v1(NKP - 1, 0, prev_e[0])
            attn_v1(NKP - 1, 1, prev_e[1])

            # evacuate + transpose + normalize; residual-add + bn_stats per
            # 128-column chunk. po banks are retired by the copies; the
            # transpose tiles ping-pong through the po tags.
            ots = []
            for p in range(2):
                ot = osb.tile([DK + 1, 512], f32, tag=f"ot{p}", name="ot")
                nc.scalar.activation(out=ot, in_=po[p], func=AF.Relu)
                ots.append(ot)
            for j in range(4):
                qi = qb * 4 + j
                for p in range(2):
                    h = 2 * hp + p
                    pt = ps_attn.tile([128, DK + 1], f32, tag=f"po{(2 * j + p) % 2}", name="ps_t")
                    nc.tensor.transpose(
                        pt, ots[p][:, 128 * j : 128 * (j + 1)], id_sb[0 : DK + 1, 0 : DK + 1]
                    )
                    rc = smalls.tile([128, 1], f32, tag="rc", name="rc")
                    nc.vector.reciprocal(rc, pt[:, DK : DK + 1])
                    nc.scalar.activation(
                        out=om_sb[qi][:, DK * h : DK * (h + 1)],
                        in_=pt[:, 0:DK],
                        func=AF.Relu,
                        scale=rc,
                    )
                cols = slice(128 * hp, 128 * (hp + 1))
                nc.gpsimd.tensor_add(
                    out=om_sb[qi][:, cols], in0=om_sb[qi][:, cols], in1=qr_sb[qi][:, cols]
                )
                nc.vector.bn_stats(out=st_sb[qi][:, hp, :], in_=om_sb[qi][:, cols])
                if hp == DT - 1:
                    layernorm(qi)

        # ---------------- layernorm finalize ----------------
        def layernorm(qi):
            mv = statp.tile([128, 2], f32, tag="mv", name="mv")
            nc.vector.bn_aggr(out=mv, in_=st_sb[qi])
            # rstd = (var*n/(n-1) + eps)^-0.5 via Ln+Exp (one ACT table set)
            lnv = statp.tile([128, 1], f32, tag="lnv", name="lnv")
            nc.scalar.activation(
                out=lnv, in_=mv[:, 1:2], func=AF.Ln, scale=float(D) / (D - 1), bias=eps_sb
            )
            rstd = statp.tile([128, 1], f32, tag="rstd", name="rstd")
            nc.scalar.activation(out=rstd, in_=lnv, func=AF.Exp, scale=-0.5)
            fo = fop.tile([128, D], bf16, tag=f"fo{qi % 2}", name="fo")
            nc.vector.tensor_scalar(
                out=fo,
                in0=om_sb[qi],
                scalar1=mv[:, 0:1],
                scalar2=rstd,
                op0=ALU.subtract,
                op1=ALU.mult,
            )
            eng = nc.vector if qi >= 4 else nc.gpsimd
            eng.tensor_mul(out=fo, in0=fo, in1=gam_sb)
            eng.tensor_add(out=fo, in0=fo, in1=bet_sb)
            nc.sync.dma_start(out=out[128 * qi : 128 * (qi + 1), :], in_=fo)

        # ---------------- emission schedule ----------------
        # Pre-attention: only the units whose DMAs land first. K-projection
        # l-block lb is consumed by scores(kp=2lb), so the remaining blocks
        # ride as fillers far enough ahead (lb2/lb3 need kT's second half,
        # which lands latest).
        for which, lb in QK_UNITS[:3]:
            proj_qk_unit(0, which, lb)

        def filler_sched(hp, qb):
            sched = [[] for _ in range(NKP)]
            if hp == 0 and qb == 0:
                for kp in range(NKP):
                    sched[kp] = [
                        (lambda t=2 * kp: proj_v_unit(t)),
                        (lambda t=2 * kp + 1: proj_v_unit(t)),
                    ]
                sched[0].insert(0, lambda: proj_qk_unit(0, "k", 1))
                sched[3].insert(0, lambda: proj_qk_unit(0, "k", 2))
                sched[4].insert(0, lambda: proj_qk_unit(0, "k", 3))
            elif hp == 0 and qb == 1:
                for u, (which, lb) in enumerate(QK_UNITS):
                    kp = min(u + 1, NKP - 1)
                    sched[kp].append(lambda w=which, l=lb: proj_qk_unit(1, w, l))
            elif hp < DT - 1:
                halfu = QK_UNITS[:3] if qb == 0 else QK_UNITS[3:]
                for u, (which, lb) in enumerate(halfu):
                    kp = min(2 * u + 1, NKP - 1)
                    sched[kp].append(lambda i=hp + 1, w=which, l=lb: proj_qk_unit(i, w, l))
            return sched

        for hp in range(DT):
            for qb in range(2):
                attention(hp, qb, filler_sched(hp, qb))


def _build():
    global _COMPILED
    if _COMPILED is not None:
        return _COMPILED
    import concourse.bacc as bacc
    import concourse.tile as tile
    from concourse import mybir

    f32 = mybir.dt.float32
    bf16 = mybir.dt.bfloat16

    # Keep Exp/Ln in one ACT table set so a single table load serves all.
    if not getattr(bacc, "_act_tables_patched", False):
        _orig_get = bacc.get_activation_tables

        def _patched(arch):
            tables = _orig_get(arch)
            AF = mybir.ActivationFunctionType
            combined = "natural_log_exp_and_others"
            if combined in tables:
                for name, funcs in tables.items():
                    if name != combined:
                        funcs.discard(AF.Exp)
                        funcs.discard(AF.Ln)
            return tables

        bacc.get_activation_tables = _patched
        bacc._act_tables_patched = True

    nc = bacc.Bacc("TRN2", target_bir_lowering=False, debug=False, num_devices=N_CORES)
    aps = (
        nc.dram_tensor("qT", [D, LQ], bf16, kind="ExternalInput").ap(),
        nc.dram_tensor("kT", [D, LK], bf16, kind="ExternalInput").ap(),
        nc.dram_tensor("qres", [LQ, D], f32, kind="ExternalInput").ap(),
        nc.dram_tensor("wqT", [D, D], bf16, kind="ExternalInput").ap(),
        nc.dram_tensor("wkT", [D, D], bf16, kind="ExternalInput").ap(),
        nc.dram_tensor("wvT", [D, D], bf16, kind="ExternalInput").ap(),
        nc.dram_tensor("bq8", [D], f32, kind="ExternalInput").ap(),
        nc.dram_tensor("bkv", [D], f32, kind="ExternalInput").ap(),
        nc.dram_tensor("bvb", [D], bf16, kind="ExternalInput").ap(),
        nc.dram_tensor("gam", [D], bf16, kind="ExternalInput").ap(),
        nc.dram_tensor("bet", [D], bf16, kind="ExternalInput").ap(),
        nc.dram_tensor("iden", [128, 128], f32, kind="ExternalInput").ap(),
        nc.dram_tensor("out", [LQ, D], bf16, kind="ExternalOutput").ap(),
    )
    with tile.TileContext(nc) as tc:
        _emit(tc, aps)
    nc.compile()
    _COMPILED = nc
    return nc


def _in_maps(inputs):
    bf = ml_dtypes.bfloat16
    q = np.asarray(inputs["query"], np.float32)
    k = np.asarray(inputs["key"], np.float32)
    sc = A16 / 8.0
    shared = {
        "wqT": np.ascontiguousarray((np.asarray(inputs["Wq"], np.float32) * sc).T).astype(bf),
        "wkT": np.ascontiguousarray(np.asarray(inputs["Wk"], np.float32).T).astype(bf),
        "wvT": np.ascontiguousarray(np.asarray(inputs["Wv"], np.float32).T).astype(bf),
        "bq8": np.asarray(inputs["bq"], np.float32) * sc,
        "bkv": np.asarray(inputs["bk"], np.float32),
        "bvb": np.asarray(inputs["bv"], np.float32).astype(bf),
        "gam": np.asarray(inputs["gamma"], np.float32).astype(bf),
        "bet": np.asarray(inputs["beta"], np.float32).astype(bf),
        "iden": np.eye(128, dtype=np.float32),
    }
    maps = []
    for c in range(N_CORES):
        b, hf = divmod(c, 2)
        qs = q[b, hf * LQ : (hf + 1) * LQ]
        maps.append(
            {
                "qT": np.ascontiguousarray(qs.T).astype(bf),
                "kT": np.ascontiguousarray(k[b].T).astype(bf),
                "qres": np.ascontiguousarray(qs),
                **shared,
            }
        )
    return maps


def _assemble(results):
    out = np.empty((B, L, D), np.float32)
    for c in range(N_CORES):
        b, hf = divmod(c, 2)
        out[b, hf * LQ : (hf + 1) * LQ] = results[c]["out"].astype(np.float32)
    return out


def kernel(**inputs) -> np.ndarray:
    from concourse.bass_utils import run_bass_kernel_spmd

    nc = _build()
    res = run_bass_kernel_spmd(nc, _in_maps(inputs), list(range(N_CORES)))
    return _assemble(res.results)


def _install_ntff_hook():
    """Make `antenv.axon_hooks` importable (the image's antenv lacks it)."""
    import importlib.util

    if "antenv.axon_hooks" in sys.modules:
        return
    spec = importlib.util.spec_from_file_location(
        "antenv.axon_hooks", "/opt/trn_rl_repo/antenv/axon_hooks.py"
    )
    mod = importlib.util.module_from_spec(spec)
    sys.modules["antenv.axon_hooks"] = mod
    spec.loader.exec_module(mod)


def run_traced(inputs, **trace_kwargs):
    """Like kernel() but with NTFF tracing; returns (out, BassKernelResults)."""
    from concourse.bass_utils import run_bass_kernel_spmd

    _install_ntff_hook()

    nc = _build()
    res = run_bass_kernel_spmd(
        nc, _in_maps(inputs), list(range(N_CORES)), trace=True, **trace_kwargs
    )
    return _assemble(res.results), res


# revision 25
# speedup vs baseline: 1.1134x; 1.1134x over previous
"""MultiHeadAttention (QKV proj + softmax attention + residual + LayerNorm)
for Trainium2, SPMD across 8 NeuronCores.

Sharding: data-parallel over (batch, query-L-half): core c handles batch c//2,
query rows [1024*(c%2), 1024*(c%2)+1024), all 12 heads, full 2048 keys.
No cross-core communication.

Structure (v10, ~371us vs 606us staged baseline):
- Paired-chunk input DMAs on the two HWDGE rings (sync/scalar), ordered by
  first use; gpsimd SWDGE only carries late-needed tensors.
- Q/K projection e-chunk 0 runs first; attention follows with the V
  projection and remaining Q/K projection chunks emitted as PE filler
  inside the attention loop.
- The attention inner loop is paced by the score-psum ping-pong cycle
  (scores -> exp -> scores, single-buffered due to the 8-bank PSUM budget).
  The two heads of a pair therefore run their exp on DIFFERENT engines
  (head0 exact exp on ScalarE; head1 Schraudolph bit-trick on VectorE:
  int8(A*s+B) bitcast to fp8e4m3), so the two cycles progress in parallel;
  attnV trails scores by one kp so nothing waits on a just-issued exp.
- attnV runs in fp8-e4m3 DoubleRow (contraction 256 = both k-chunks of a
  kp in one matmul). exp outputs are fp8 with a -4 shift (softmax is
  shift-invariant; keeps e^s inside e4m3 range, max score ~8.5).
- Out-path copies/normalize run on ScalarE (attn outputs are >=0, so Relu
  with per-partition scale=1/denominator is an exact normalize), keeping
  VectorE free for its exp share. Residual-add + bn_stats run inline per
  128-column chunk; the layernorm tail only aggregates + scales. Final
  output bf16 (restored to f32 on host).

Numerics: projections/scores in bf16 (fp32 accumulate), attnV fp8,
normalization + layernorm f32, residual/gamma/beta/output bf16. Scale
A16/8 (A16=2^7/ln2) is folded into Wq/bq on the host. End-to-end rel err
~6e-3 vs the 2e-2 gate (validated in numpy sim + on HW).
"""

import sys

sys.path.insert(0, "/opt/trn_rl_repo")

import numpy as np
import ml_dtypes

N_CORES = 8
B, L, D = 4, 2048, 768
H, DK = 12, 64
LQ = L // 2  # 1024 query rows per core
LK = L  # full keys per core
DT = D // 128  # 6 d-chunks
NKC = LK // 128  # 16 k-chunks
NKP = NKC // 2  # 8 kp iterations (2 k-chunks each)
VH = 80  # per-head stride in the fp8 V tile (16B-aligned)

A16 = 128.0 / float(np.log(2.0))  # folded score scale (2^7/ln2)
CSH = 4.0  # softmax shift: exp(s - CSH)
A8 = 8.0 / float(np.log(2.0))  # e4m3 Schraudolph scale
B8 = 56.0 - A8 * CSH - 0.47  # e4m3 exponent bias - shift - mid correction


def _use_dve_exp(hp, qb, kp, p):
    """Head p1's exp runs on VectorE (Schraudolph) so the two score-psum
    ping-pong cycles pace on different engines in parallel."""
    return p == 1


_COMPILED = None


def _emit(tc, aps):
    import contextlib

    import concourse.bass as bass
    from concourse import mybir

    nc = tc.nc
    f32 = mybir.dt.float32
    bf16 = mybir.dt.bfloat16
    fp8 = mybir.dt.float8e4
    i8 = mybir.dt.int8
    AF = mybir.ActivationFunctionType
    ALU = mybir.AluOpType
    PM = mybir.MatmulPerfMode

    qT, kT, qres, wqT, wkT, wvT, bq8, bkv, bvb, gam, bet, iden, out = aps

    ctx = contextlib.ExitStack()
    with ctx:
        const = ctx.enter_context(tc.tile_pool(name="const", bufs=1))
        persist = ctx.enter_context(tc.tile_pool(name="persist", bufs=1))
        # PSUM (8 banks of 2KB): s0,s1 [128,1024]f32 = 2 banks each; po0,po1
        # [65,512] 1 bank each (transpose tiles reuse them); pj0,pj1
        # (projection accumulators, ping-pong) 1 bank each.
        ps_score = ctx.enter_context(tc.tile_pool(name="ps_score", bufs=1, space="PSUM"))
        ps_attn = ctx.enter_context(tc.tile_pool(name="ps_attn", bufs=1, space="PSUM"))
        ps_proj = ctx.enter_context(tc.tile_pool(name="ps_proj", bufs=1, space="PSUM"))
        expp = ctx.enter_context(tc.tile_pool(name="expp", bufs=6))
        osb = ctx.enter_context(tc.tile_pool(name="osb", bufs=2))
        smalls = ctx.enter_context(tc.tile_pool(name="smalls", bufs=4))
        qrp = ctx.enter_context(tc.tile_pool(name="qrp", bufs=1))
        statp = ctx.enter_context(tc.tile_pool(name="statp", bufs=4))
        fop = ctx.enter_context(tc.tile_pool(name="fop", bufs=2))

        # ---------------- input tiles ----------------
        # Paired-chunk dma_starts (amortize per-transfer fixed cost) spread
        # round-robin over four issue queues; order = first use.
        _rings = [nc.sync, nc.scalar]
        _ring_rr = [0]

        def _dma(out_ap, in_ap):
            eng = _rings[_ring_rr[0] % len(_rings)]
            _ring_rr[0] += 1
            eng.dma_start(out=out_ap, in_=in_ap)

        def big_tile(name, ncols):
            t = const.tile([128, DT, ncols], bf16, tag=name, name=name)
            return t, [t[:, i, :] for i in range(DT)]

        wq_t, wq_sb = big_tile("wqc", D)
        qT_t, qT_sb = big_tile("qTc", LQ)
        wk_t, wk_sb = big_tile("wkc", D)
        kT_t, kT_sb = big_tile("kTc", LK)
        wv_t, wv_sb = big_tile("wvc", D)

        def pair_dmas(t, dram, ncols, c0=0, c1=None):
            for i in range(DT // 2):
                src = bass.AP(
                    tensor=dram.tensor,
                    offset=dram.offset + 256 * i * ncols + c0,
                    ap=[[ncols, 128], [128 * ncols, 2], [1, (c1 or ncols) - c0]],
                )
                _dma(t[:, 2 * i : 2 * i + 2, c0 : c1 or ncols], src)

        pair_dmas(wq_t, wqT, D)
        pair_dmas(qT_t, qT, LQ)
        pair_dmas(wk_t, wkT, D)
        pair_dmas(kT_t, kT, LK, 0, 1024)
        pair_dmas(wv_t, wvT, D)
        pair_dmas(kT_t, kT, LK, 1024, 2048)

        def load_bias(name, dram, eng):
            t = const.tile([128, DT, 1], f32, tag=name, name=name)
            src = bass.AP(
                tensor=dram.tensor, offset=dram.offset, ap=[[1, 128], [128, DT], [0, 1]]
            )
            eng.dma_start(out=t, in_=src)
            return [t[:, i, :] for i in range(DT)]

        bq_sb = load_bias("bq", bq8, nc.sync)
        bk_sb = load_bias("bk", bkv, nc.scalar)
        bv_sb = const.tile([1, D], bf16, tag="bv", name="bv_sb")
        nc.sync.dma_start(out=bv_sb, in_=bvb[:])
        id_sb = const.tile([65, 65], f32, tag="iden", name="id_sb")
        nc.gpsimd.dma_start(out=id_sb, in_=iden[0:65, 0:65])
        gam_sb = const.tile([128, D], bf16, tag="gam", name="gam_sb")
        nc.gpsimd.dma_start(
            out=gam_sb,
            in_=bass.AP(tensor=gam.tensor, offset=gam.offset, ap=[[0, 128]] + list(gam.ap)),
        )
        bet_sb = const.tile([128, D], bf16, tag="bet", name="bet_sb")
        nc.gpsimd.dma_start(
            out=bet_sb,
            in_=bass.AP(tensor=bet.tensor, offset=bet.offset, ap=[[0, 128]] + list(bet.ap)),
        )
        ones_sb = const.tile([1, 128], bf16, tag="ones", name="ones_sb")
        nc.vector.memset(ones_sb, 1.0)
        eps_sb = const.tile([128, 1], f32, tag="eps", name="eps_sb")
        nc.vector.memset(eps_sb, 1e-5)
        csh_sb = const.tile([128, 1], f32, tag="csh", name="csh_sb")
        nc.vector.memset(csh_sb, -CSH)

        # residual tiles (first needed ~45us in)
        qr_sb = [qrp.tile([128, D], bf16, tag=f"qr{qi}", name=f"qr{qi}") for qi in range(LQ // 128)]
        for qi in range(LQ // 128):
            (nc.gpsimd if qi % 2 else nc.sync).dma_start(
                out=qr_sb[qi], in_=qres[128 * qi : 128 * (qi + 1), :]
            )

        # persistent intermediates
        pq_sb = [persist.tile([128, LQ], bf16, tag=f"pq{i}", name=f"pq_sb{i}") for i in range(DT)]
        pk_sb = [persist.tile([128, LK], bf16, tag=f"pk{i}", name=f"pk_sb{i}") for i in range(DT)]
        # V in fp8, kc pairs interleaved for DoubleRow: [l-part, pair, head, VH]
        v_sb = [
            persist.tile([128, 2, H, VH], fp8, tag=f"v{t}", name=f"v_sb{t}")
            for t in range(NKP)
        ]
        om_sb = [persist.tile([128, D], f32, tag=f"om{q}", name=f"om_sb{q}") for q in range(LQ // 128)]
        st_sb = [persist.tile([128, DT, 6], f32, tag=f"st{q}", name=f"st_sb{q}") for q in range(LQ // 128)]

        # ---------------- projection units (emitted as PE filler) ----------
        _pj_rr = [0]

        def _pj_tile():
            ps = ps_proj.tile([128, 512], f32, tag=f"pj{_pj_rr[0] % 2}", name="ps_p")
            _pj_rr[0] += 1
            return ps

        def proj_qk_unit(i, which, lb):
            src_tiles, w_tiles, b_tiles, dst = (
                (qT_sb, wq_sb, bq_sb, pq_sb) if which == "q" else (kT_sb, wk_sb, bk_sb, pk_sb)
            )
            ps = _pj_tile()
            for kc in range(DT):
                nc.tensor.matmul(
                    ps,
                    lhsT=w_tiles[kc][:, 128 * i : 128 * (i + 1)],
                    rhs=src_tiles[kc][:, 512 * lb : 512 * (lb + 1)],
                    start=kc == 0,
                    stop=kc == DT - 1,
                )
            nc.scalar.activation(
                out=dst[i][:, 512 * lb : 512 * (lb + 1)],
                in_=ps,
                func=AF.Relu,
                bias=b_tiles[i],
            )

        QK_UNITS = [("q", 0), ("q", 1), ("k", 0), ("k", 1), ("k", 2), ("k", 3)]

        def proj_v_unit(t):
            kp2, sl = divmod(t, 2)
            if sl == 0:
                nc.vector.memset(v_sb[kp2][:, :, :, DK : DK + 1], 1.0)
            for ei, (e0, ew) in enumerate(((0, 512), (512, 256))):
                ps = _pj_tile()
                for kc in range(DT):
                    nc.tensor.matmul(
                        ps[:, 0:ew],
                        lhsT=kT_sb[kc][:, 128 * t : 128 * (t + 1)],
                        rhs=wv_sb[kc][:, e0 : e0 + ew],
                        start=kc == 0,
                        stop=False,
                    )
                nc.tensor.matmul(
                    ps[:, 0:ew],
                    lhsT=ones_sb[:, 0:128],
                    rhs=bv_sb[:, e0 : e0 + ew],
                    start=False,
                    stop=True,
                )
                nc.vector.tensor_scalar(
                    out=v_sb[kp2][:, sl, e0 // DK : (e0 + ew) // DK, 0:DK],
                    in0=ps[:, 0:ew].rearrange("p (h x) -> p h x", x=DK),
                    scalar1=0.0,
                    scalar2=None,
                    op0=ALU.max,
                )

        # ---------------- attention ----------------
        def attention(hp, qb, filler):
            po = [
                ps_attn.tile([DK + 1, 512], f32, tag=f"po{p}", name=f"ps_o{p}")
                for p in range(2)
            ]

            def attn_v1(kp, p, e_t):
                # one fp8 DoubleRow matmul: contraction over both k-chunks
                # of kp (pair dim), accumulating into po[p].
                h = 2 * hp + p
                nc.tensor.matmul(
                    po[p],
                    lhsT=v_sb[kp][:, :, h, 0 : DK + 1],
                    rhs=e_t.rearrange("x (two q) -> x two q", two=2),
                    start=kp == 0,
                    stop=kp == NKP - 1,
                    perf_mode=PM.DoubleRow,
                )

            def scores(kp, p, ps):
                # scores^T [k-part, q-free]; head 2hp+p row-tiled (K=64)
                for half in range(2):
                    kc = 2 * kp + half
                    nc.tensor.matmul(
                        ps[:, 512 * half : 512 * (half + 1)],
                        lhsT=pk_sb[hp][64 * p : 64 * (p + 1), 128 * kc : 128 * (kc + 1)],
                        rhs=pq_sb[hp][64 * p : 64 * (p + 1), 512 * qb : 512 * (qb + 1)],
                        start=True,
                        stop=True,
                        tile_position=(64 * p, 0),
                    )

            def exp(kp, p, ps):
                e_t = expp.tile([128, 1024], fp8, tag="exp", bufs=6, name="e_t")
                if _use_dve_exp(hp, qb, kp, p):
                    nc.vector.tensor_scalar(
                        out=e_t.bitcast(i8),
                        in0=ps,
                        scalar1=1.0 / 16.0,
                        scalar2=B8,
                        op0=ALU.mult,
                        op1=ALU.add,
                    )
                else:
                    nc.scalar.activation(
                        out=e_t, in_=ps, func=AF.Exp, scale=1.0 / A16, bias=csh_sb
                    )
                return e_t

            # Software pipeline: each head's score matmuls sit ~one block
            # after the exp that frees their psum tile, so nothing stalls;
            # attnV for kp-1 consumes the previous block's exp output.
            prev_e = [None, None]
            for kp in range(NKP):
                pspair = [
                    ps_score.tile([128, 1024], f32, tag=f"s{p}", name=f"ps_s{p}")
                    for p in range(2)
                ]
                scores(kp, 0, pspair[0])
                e0 = exp(kp, 0, pspair[0])
                if kp > 0:
                    attn_v1(kp - 1, 0, prev_e[0])
                scores(kp, 1, pspair[1])
                e1 = exp(kp, 1, pspair[1])
                if kp > 0:
                    attn_v1(kp - 1, 1, prev_e[1])
                for f in filler[kp]:
                    f()
                prev_e = [e0, e1]
            attn_v1(NKP - 1, 0, prev_e[0])
            attn_v1(NKP - 1, 1, prev_e[1])

            # evacuate + transpose + normalize; residual-add + bn_stats per
            # 128-column chunk. po banks are retired by the copies; the
            # transpose tiles ping-pong through the po tags.
            ots = []
            for p in range(2):
                ot = osb.tile([DK + 1, 512], f32, tag=f"ot{p}", name="ot")
                nc.scalar.activation(out=ot, in_=po[p], func=AF.Relu)
                ots.append(ot)
            for j in range(4):
                qi = qb * 4 + j
                for p in range(2):
                    h = 2 * hp + p
                    pt = ps_attn.tile([128, DK + 1], f32, tag=f"po{(2 * j + p) % 2}", name="ps_t")
                    nc.tensor.transpose(
                        pt, ots[p][:, 128 * j : 128 * (j + 1)], id_sb[0 : DK + 1, 0 : DK + 1]
                    )
                    rc = smalls.tile([128, 1], f32, tag="rc", name="rc")
                    nc.vector.reciprocal(rc, pt[:, DK : DK + 1])
                    nc.scalar.activation(
                        out=om_sb[qi][:, DK * h : DK * (h + 1)],
                        in_=pt[:, 0:DK],
                        func=AF.Relu,
                        scale=rc,
                    )
                cols = slice(128 * hp, 128 * (hp + 1))
                nc.vector.tensor_add(
                    out=om_sb[qi][:, cols], in0=om_sb[qi][:, cols], in1=qr_sb[qi][:, cols]
                )
                nc.vector.bn_stats(out=st_sb[qi][:, hp, :], in_=om_sb[qi][:, cols])
                if hp == DT - 1:
                    layernorm(qi)

        # ---------------- layernorm finalize ----------------
        def layernorm(qi):
            mv = statp.tile([128, 2], f32, tag="mv", name="mv")
            nc.vector.bn_aggr(out=mv, in_=st_sb[qi])
            # rstd = (var*n/(n-1) + eps)^-0.5 via Ln+Exp (one ACT table set)
            lnv = statp.tile([128, 1], f32, tag="lnv", name="lnv")
            nc.scalar.activation(
                out=lnv, in_=mv[:, 1:2], func=AF.Ln, scale=float(D) / (D - 1), bias=eps_sb
            )
            rstd = statp.tile([128, 1], f32, tag="rstd", name="rstd")
            nc.scalar.activation(out=rstd, in_=lnv, func=AF.Exp, scale=-0.5)
            fo = fop.tile([128, D], bf16, tag=f"fo{qi % 2}", name="fo")
            nc.vector.tensor_scalar(
                out=fo,
                in0=om_sb[qi],
                scalar1=mv[:, 0:1],
                scalar2=rstd,
                op0=ALU.subtract,
                op1=ALU.mult,
            )
            nc.gpsimd.tensor_mul(out=fo, in0=fo, in1=gam_sb)
            nc.gpsimd.tensor_add(out=fo, in0=fo, in1=bet_sb)
            nc.sync.dma_start(out=out[128 * qi : 128 * (qi + 1), :], in_=fo)

        # ---------------- emission schedule ----------------
        # Pre-attention: only the units whose DMAs land first. K-projection
        # l-block lb is consumed by scores(kp=2lb), so the remaining blocks
        # ride as fillers far enough ahead (lb2/lb3 need kT's second half,
        # which lands latest).
        for which, lb in QK_UNITS[:3]:
            proj_qk_unit(0, which, lb)

        def filler_sched(hp, qb):
            sched = [[] for _ in range(NKP)]
            if hp == 0 and qb == 0:
                for kp in range(NKP):
                    sched[kp] = [
                        (lambda t=2 * kp: proj_v_unit(t)),
                        (lambda t=2 * kp + 1: proj_v_unit(t)),
                    ]
                sched[0].insert(0, lambda: proj_qk_unit(0, "k", 1))
                sched[3].insert(0, lambda: proj_qk_unit(0, "k", 2))
                sched[4].insert(0, lambda: proj_qk_unit(0, "k", 3))
            elif hp == 0 and qb == 1:
                for u, (which, lb) in enumerate(QK_UNITS):
                    kp = min(u + 1, NKP - 1)
                    sched[kp].append(lambda w=which, l=lb: proj_qk_unit(1, w, l))
            elif hp < DT - 1:
                halfu = QK_UNITS[:3] if qb == 0 else QK_UNITS[3:]
                for u, (which, lb) in enumerate(halfu):
                    kp = min(2 * u + 1, NKP - 1)
                    sched[kp].append(lambda i=hp + 1, w=which, l=lb: proj_qk_unit(i, w, l))
            return sched

        for hp in range(DT):
            for qb in range(2):
                attention(hp, qb, filler_sched(hp, qb))


def _build():
    global _COMPILED
    if _COMPILED is not None:
        return _COMPILED
    import concourse.bacc as bacc
    import concourse.tile as tile
    from concourse import mybir

    f32 = mybir.dt.float32
    bf16 = mybir.dt.bfloat16

    # Keep Exp/Ln in one ACT table set so a single table load serves all.
    if not getattr(bacc, "_act_tables_patched", False):
        _orig_get = bacc.get_activation_tables

        def _patched(arch):
            tables = _orig_get(arch)
            AF = mybir.ActivationFunctionType
            combined = "natural_log_exp_and_others"
            if combined in tables:
                for name, funcs in tables.items():
                    if name != combined:
                        funcs.discard(AF.Exp)
                        funcs.discard(AF.Ln)
            return tables

        bacc.get_activation_tables = _patched
        bacc._act_tables_patched = True

    nc = bacc.Bacc("TRN2", target_bir_lowering=False, debug=False, num_devices=N_CORES)
    aps = (
        nc.dram_tensor("qT", [D, LQ], bf16, kind="ExternalInput").ap(),
        nc.dram_tensor("kT", [D, LK], bf16, kind="ExternalInput").ap(),
        nc.dram_tensor("qres", [LQ, D], bf16, kind="ExternalInput").ap(),
        nc.dram_tensor("wqT", [D, D], bf16, kind="ExternalInput").ap(),
        nc.dram_tensor("wkT", [D, D], bf16, kind="ExternalInput").ap(),
        nc.dram_tensor("wvT", [D, D], bf16, kind="ExternalInput").ap(),
        nc.dram_tensor("bq8", [D], f32, kind="ExternalInput").ap(),
        nc.dram_tensor("bkv", [D], f32, kind="ExternalInput").ap(),
        nc.dram_tensor("bvb", [D], bf16, kind="ExternalInput").ap(),
        nc.dram_tensor("gam", [D], bf16, kind="ExternalInput").ap(),
        nc.dram_tensor("bet", [D], bf16, kind="ExternalInput").ap(),
        nc.dram_tensor("iden", [128, 128], f32, kind="ExternalInput").ap(),
        nc.dram_tensor("out", [LQ, D], bf16, kind="ExternalOutput").ap(),
    )
    with tile.TileContext(nc) as tc:
        _emit(tc, aps)
    nc.compile()
    _COMPILED = nc
    return nc


def _in_maps(inputs):
    bf = ml_dtypes.bfloat16
    q = np.asarray(inputs["query"], np.float32)
    k = np.asarray(inputs["key"], np.float32)
    sc = A16 / 8.0
    shared = {
        "wqT": np.ascontiguousarray((np.asarray(inputs["Wq"], np.float32) * sc).T).astype(bf),
        "wkT": np.ascontiguousarray(np.asarray(inputs["Wk"], np.float32).T).astype(bf),
        "wvT": np.ascontiguousarray(np.asarray(inputs["Wv"], np.float32).T).astype(bf),
        "bq8": np.asarray(inputs["bq"], np.float32) * sc,
        "bkv": np.asarray(inputs["bk"], np.float32),
        "bvb": np.asarray(inputs["bv"], np.float32).astype(bf),
        "gam": np.asarray(inputs["gamma"], np.float32).astype(bf),
        "bet": np.asarray(inputs["beta"], np.float32).astype(bf),
        "iden": np.eye(128, dtype=np.float32),
    }
    maps = []
    for c in range(N_CORES):
        b, hf = divmod(c, 2)
        qs = q[b, hf * LQ : (hf + 1) * LQ]
        maps.append(
            {
                "qT": np.ascontiguousarray(qs.T).astype(bf),
                "kT": np.ascontiguousarray(k[b].T).astype(bf),
                "qres": np.ascontiguousarray(qs).astype(bf),
                **shared,
            }
        )
    return maps


def _assemble(results):
    out = np.empty((B, L, D), np.float32)
    for c in range(N_CORES):
        b, hf = divmod(c, 2)
        out[b, hf * LQ : (hf + 1) * LQ] = results[c]["out"].astype(np.float32)
    return out


def kernel(**inputs) -> np.ndarray:
    from concourse.bass_utils import run_bass_kernel_spmd

    nc = _build()
    res = run_bass_kernel_spmd(nc, _in_maps(inputs), list(range(N_CORES)))
    return _assemble(res.results)


def _install_ntff_hook():
    """Make `antenv.axon_hooks` importable (the image's antenv lacks it)."""
    import importlib.util

    if "antenv.axon_hooks" in sys.modules:
        return
    spec = importlib.util.spec_from_file_location(
        "antenv.axon_hooks", "/opt/trn_rl_repo/antenv/axon_hooks.py"
    )
    mod = importlib.util.module_from_spec(spec)
    sys.modules["antenv.axon_hooks"] = mod
    spec.loader.exec_module(mod)


def run_traced(inputs, **trace_kwargs):
    """Like kernel() but with NTFF tracing; returns (out, BassKernelResults)."""
    from concourse.bass_utils import run_bass_kernel_spmd

    _install_ntff_hook()

    nc = _build()
    res = run_bass_kernel_spmd(
        nc, _in_maps(inputs), list(range(N_CORES)), trace=True, **trace_kwargs
    )
    return _assemble(res.results), res


# revision 26
# speedup vs baseline: 1.1224x; 1.0081x over previous
"""MultiHeadAttention (QKV proj + softmax attention + residual + LayerNorm)
for Trainium2, SPMD across 8 NeuronCores.

Sharding: data-parallel over (batch, query-L-half): core c handles batch c//2,
query rows [1024*(c%2), 1024*(c%2)+1024), all 12 heads, full 2048 keys.
No cross-core communication.

Structure (v10, ~371us vs 606us staged baseline):
- Paired-chunk input DMAs on the two HWDGE rings (sync/scalar), ordered by
  first use; gpsimd SWDGE only carries late-needed tensors.
- Q/K projection e-chunk 0 runs first; attention follows with the V
  projection and remaining Q/K projection chunks emitted as PE filler
  inside the attention loop.
- The attention inner loop is paced by the score-psum ping-pong cycle
  (scores -> exp -> scores, single-buffered due to the 8-bank PSUM budget).
  The two heads of a pair therefore run their exp on DIFFERENT engines
  (head0 exact exp on ScalarE; head1 Schraudolph bit-trick on VectorE:
  int8(A*s+B) bitcast to fp8e4m3), so the two cycles progress in parallel;
  attnV trails scores by one kp so nothing waits on a just-issued exp.
- attnV runs in fp8-e4m3 DoubleRow (contraction 256 = both k-chunks of a
  kp in one matmul). exp outputs are fp8 with a -4 shift (softmax is
  shift-invariant; keeps e^s inside e4m3 range, max score ~8.5).
- Out-path copies/normalize run on ScalarE (attn outputs are >=0, so Relu
  with per-partition scale=1/denominator is an exact normalize), keeping
  VectorE free for its exp share. Residual-add + bn_stats run inline per
  128-column chunk; the layernorm tail only aggregates + scales. Final
  output bf16 (restored to f32 on host).

Numerics: projections/scores in bf16 (fp32 accumulate), attnV fp8,
normalization + layernorm f32, residual/gamma/beta/output bf16. Scale
A16/8 (A16=2^7/ln2) is folded into Wq/bq on the host. End-to-end rel err
~6e-3 vs the 2e-2 gate (validated in numpy sim + on HW).
"""

import sys

sys.path.insert(0, "/opt/trn_rl_repo")

import numpy as np
import ml_dtypes

N_CORES = 8
B, L, D = 4, 2048, 768
H, DK = 12, 64
LQ = L // 2  # 1024 query rows per core
LK = L  # full keys per core
DT = D // 128  # 6 d-chunks
NKC = LK // 128  # 16 k-chunks
NKP = NKC // 2  # 8 kp iterations (2 k-chunks each)
VH = 80  # per-head stride in the fp8 V tile (16B-aligned)

A16 = 128.0 / float(np.log(2.0))  # folded score scale (2^7/ln2)
CSH = 4.0  # softmax shift: exp(s - CSH)
A8 = 8.0 / float(np.log(2.0))  # e4m3 Schraudolph scale
B8 = 56.0 - A8 * CSH - 0.47  # e4m3 exponent bias - shift - mid correction


def _use_dve_exp(hp, qb, kp, p):
    """Head p1's exp runs on VectorE (Schraudolph) so the two score-psum
    ping-pong cycles pace on different engines in parallel."""
    return p == 1


_COMPILED = None


def _emit(tc, aps):
    import contextlib

    import concourse.bass as bass
    from concourse import mybir

    nc = tc.nc
    f32 = mybir.dt.float32
    bf16 = mybir.dt.bfloat16
    fp8 = mybir.dt.float8e4
    i8 = mybir.dt.int8
    AF = mybir.ActivationFunctionType
    ALU = mybir.AluOpType
    PM = mybir.MatmulPerfMode

    qT, kT, qres, wqT, wkT, wvT, bq8, bkv, bvb, gam, bet, iden, out = aps

    ctx = contextlib.ExitStack()
    with ctx:
        const = ctx.enter_context(tc.tile_pool(name="const", bufs=1))
        persist = ctx.enter_context(tc.tile_pool(name="persist", bufs=1))
        # PSUM (8 banks of 2KB): s0,s1 [128,1024]f32 = 2 banks each; po0,po1
        # [65,512] 1 bank each (transpose tiles reuse them); pj0,pj1
        # (projection accumulators, ping-pong) 1 bank each.
        ps_score = ctx.enter_context(tc.tile_pool(name="ps_score", bufs=1, space="PSUM"))
        ps_attn = ctx.enter_context(tc.tile_pool(name="ps_attn", bufs=1, space="PSUM"))
        ps_proj = ctx.enter_context(tc.tile_pool(name="ps_proj", bufs=1, space="PSUM"))
        expp = ctx.enter_context(tc.tile_pool(name="expp", bufs=6))
        osb = ctx.enter_context(tc.tile_pool(name="osb", bufs=2))
        smalls = ctx.enter_context(tc.tile_pool(name="smalls", bufs=4))
        qrp = ctx.enter_context(tc.tile_pool(name="qrp", bufs=1))
        statp = ctx.enter_context(tc.tile_pool(name="statp", bufs=4))
        fop = ctx.enter_context(tc.tile_pool(name="fop", bufs=2))

        # ---------------- input tiles ----------------
        # Paired-chunk dma_starts (amortize per-transfer fixed cost) spread
        # round-robin over four issue queues; order = first use.
        _rings = [nc.sync, nc.scalar]
        _ring_rr = [0]

        def _dma(out_ap, in_ap):
            eng = _rings[_ring_rr[0] % len(_rings)]
            _ring_rr[0] += 1
            eng.dma_start(out=out_ap, in_=in_ap)

        def big_tile(name, ncols):
            t = const.tile([128, DT, ncols], bf16, tag=name, name=name)
            return t, [t[:, i, :] for i in range(DT)]

        wq_t, wq_sb = big_tile("wqc", D)
        qT_t, qT_sb = big_tile("qTc", LQ)
        wk_t, wk_sb = big_tile("wkc", D)
        kT_t, kT_sb = big_tile("kTc", LK)
        wv_t, wv_sb = big_tile("wvc", D)

        def pair_dmas(t, dram, ncols, c0=0, c1=None):
            for i in range(DT // 2):
                src = bass.AP(
                    tensor=dram.tensor,
                    offset=dram.offset + 256 * i * ncols + c0,
                    ap=[[ncols, 128], [128 * ncols, 2], [1, (c1 or ncols) - c0]],
                )
                _dma(t[:, 2 * i : 2 * i + 2, c0 : c1 or ncols], src)

        pair_dmas(wq_t, wqT, D)
        pair_dmas(qT_t, qT, LQ)
        pair_dmas(wk_t, wkT, D)
        pair_dmas(kT_t, kT, LK, 0, 1024)
        pair_dmas(wv_t, wvT, D)
        pair_dmas(kT_t, kT, LK, 1024, 2048)

        def load_bias(name, dram, eng):
            t = const.tile([128, DT, 1], f32, tag=name, name=name)
            src = bass.AP(
                tensor=dram.tensor, offset=dram.offset, ap=[[1, 128], [128, DT], [0, 1]]
            )
            eng.dma_start(out=t, in_=src)
            return [t[:, i, :] for i in range(DT)]

        bq_sb = load_bias("bq", bq8, nc.sync)
        bk_sb = load_bias("bk", bkv, nc.scalar)
        bv_sb = const.tile([1, D], bf16, tag="bv", name="bv_sb")
        nc.sync.dma_start(out=bv_sb, in_=bvb[:])
        id_sb = const.tile([65, 65], f32, tag="iden", name="id_sb")
        nc.gpsimd.dma_start(out=id_sb, in_=iden[0:65, 0:65])
        gam_sb = const.tile([128, D], bf16, tag="gam", name="gam_sb")
        nc.gpsimd.dma_start(
            out=gam_sb,
            in_=bass.AP(tensor=gam.tensor, offset=gam.offset, ap=[[0, 128]] + list(gam.ap)),
        )
        bet_sb = const.tile([128, D], bf16, tag="bet", name="bet_sb")
        nc.gpsimd.dma_start(
            out=bet_sb,
            in_=bass.AP(tensor=bet.tensor, offset=bet.offset, ap=[[0, 128]] + list(bet.ap)),
        )
        ones_sb = const.tile([1, 128], bf16, tag="ones", name="ones_sb")
        nc.vector.memset(ones_sb, 1.0)
        eps_sb = const.tile([128, 1], f32, tag="eps", name="eps_sb")
        nc.vector.memset(eps_sb, 1e-5)
        csh_sb = const.tile([128, 1], f32, tag="csh", name="csh_sb")
        nc.vector.memset(csh_sb, -CSH)

        # residual tiles (first needed ~45us in)
        qr_sb = [qrp.tile([128, D], bf16, tag=f"qr{qi}", name=f"qr{qi}") for qi in range(LQ // 128)]
        for qi in range(LQ // 128):
            (nc.gpsimd if qi % 2 else nc.sync).dma_start(
                out=qr_sb[qi], in_=qres[128 * qi : 128 * (qi + 1), :]
            )

        # persistent intermediates
        pq_sb = [persist.tile([128, LQ], bf16, tag=f"pq{i}", name=f"pq_sb{i}") for i in range(DT)]
        pk_sb = [persist.tile([128, LK], bf16, tag=f"pk{i}", name=f"pk_sb{i}") for i in range(DT)]
        # V in fp8, kc pairs interleaved for DoubleRow: [l-part, pair, head, VH]
        v_sb = [
            persist.tile([128, 2, H, VH], fp8, tag=f"v{t}", name=f"v_sb{t}")
            for t in range(NKP)
        ]
        om_sb = [persist.tile([128, D], f32, tag=f"om{q}", name=f"om_sb{q}") for q in range(LQ // 128)]
        st_sb = [persist.tile([128, DT, 6], f32, tag=f"st{q}", name=f"st_sb{q}") for q in range(LQ // 128)]

        # ---------------- projection units (emitted as PE filler) ----------
        _pj_rr = [0]

        def _pj_tile():
            ps = ps_proj.tile([128, 512], f32, tag=f"pj{_pj_rr[0] % 2}", name="ps_p")
            _pj_rr[0] += 1
            return ps

        def proj_qk_unit(i, which, lb):
            src_tiles, w_tiles, b_tiles, dst = (
                (qT_sb, wq_sb, bq_sb, pq_sb) if which == "q" else (kT_sb, wk_sb, bk_sb, pk_sb)
            )
            ps = _pj_tile()
            for kc in range(DT):
                nc.tensor.matmul(
                    ps,
                    lhsT=w_tiles[kc][:, 128 * i : 128 * (i + 1)],
                    rhs=src_tiles[kc][:, 512 * lb : 512 * (lb + 1)],
                    start=kc == 0,
                    stop=kc == DT - 1,
                )
            nc.scalar.activation(
                out=dst[i][:, 512 * lb : 512 * (lb + 1)],
                in_=ps,
                func=AF.Relu,
                bias=b_tiles[i],
            )

        QK_UNITS = [("q", 0), ("q", 1), ("k", 0), ("k", 1), ("k", 2), ("k", 3)]

        def proj_v_unit(t):
            kp2, sl = divmod(t, 2)
            if sl == 0:
                nc.vector.memset(v_sb[kp2][:, :, :, DK : DK + 1], 1.0)
            for ei, (e0, ew) in enumerate(((0, 512), (512, 256))):
                ps = _pj_tile()
                for kc in range(DT):
                    nc.tensor.matmul(
                        ps[:, 0:ew],
                        lhsT=kT_sb[kc][:, 128 * t : 128 * (t + 1)],
                        rhs=wv_sb[kc][:, e0 : e0 + ew],
                        start=kc == 0,
                        stop=False,
                    )
                nc.tensor.matmul(
                    ps[:, 0:ew],
                    lhsT=ones_sb[:, 0:128],
                    rhs=bv_sb[:, e0 : e0 + ew],
                    start=False,
                    stop=True,
                )
                nc.vector.tensor_scalar(
                    out=v_sb[kp2][:, sl, e0 // DK : (e0 + ew) // DK, 0:DK],
                    in0=ps[:, 0:ew].rearrange("p (h x) -> p h x", x=DK),
                    scalar1=0.0,
                    scalar2=None,
                    op0=ALU.max,
                )

        # ---------------- attention ----------------
        def attention(hp, qb, filler):
            po = [
                ps_attn.tile([DK + 1, 512], f32, tag=f"po{p}", name=f"ps_o{p}")
                for p in range(2)
            ]

            def attn_v1(kp, p, e_t):
                # one fp8 DoubleRow matmul: contraction over both k-chunks
                # of kp (pair dim), accumulating into po[p].
                h = 2 * hp + p
                nc.tensor.matmul(
                    po[p],
                    lhsT=v_sb[kp][:, :, h, 0 : DK + 1],
                    rhs=e_t.rearrange("x (two q) -> x two q", two=2),
                    start=kp == 0,
                    stop=kp == NKP - 1,
                    perf_mode=PM.DoubleRow,
                )

            def scores(kp, p, ps):
                # scores^T [k-part, q-free]; head 2hp+p row-tiled (K=64)
                for half in range(2):
                    kc = 2 * kp + half
                    nc.tensor.matmul(
                        ps[:, 512 * half : 512 * (half + 1)],
                        lhsT=pk_sb[hp][64 * p : 64 * (p + 1), 128 * kc : 128 * (kc + 1)],
                        rhs=pq_sb[hp][64 * p : 64 * (p + 1), 512 * qb : 512 * (qb + 1)],
                        start=True,
                        stop=True,
                        tile_position=(64 * p, 0),
                    )

            def exp(kp, p, ps):
                e_t = expp.tile([128, 1024], fp8, tag="exp", bufs=6, name="e_t")
                if _use_dve_exp(hp, qb, kp, p):
                    nc.vector.tensor_scalar(
                        out=e_t.bitcast(i8),
                        in0=ps,
                        scalar1=1.0 / 16.0,
                        scalar2=B8,
                        op0=ALU.mult,
                        op1=ALU.add,
                    )
                else:
                    nc.scalar.activation(
                        out=e_t, in_=ps, func=AF.Exp, scale=1.0 / A16, bias=csh_sb
                    )
                return e_t

            # Software pipeline: each head's score matmuls sit ~one block
            # after the exp that frees their psum tile, so nothing stalls;
            # attnV for kp-1 consumes the previous block's exp output.
            prev_e = [None, None]
            for kp in range(NKP):
                pspair = [
                    ps_score.tile([128, 1024], f32, tag=f"s{p}", name=f"ps_s{p}")
                    for p in range(2)
                ]
                scores(kp, 0, pspair[0])
                e0 = exp(kp, 0, pspair[0])
                if kp > 0:
                    attn_v1(kp - 1, 0, prev_e[0])
                scores(kp, 1, pspair[1])
                e1 = exp(kp, 1, pspair[1])
                if kp > 0:
                    attn_v1(kp - 1, 1, prev_e[1])
                for f in filler[kp]:
                    f()
                prev_e = [e0, e1]
            attn_v1(NKP - 1, 0, prev_e[0])
            attn_v1(NKP - 1, 1, prev_e[1])

            # evacuate + transpose + normalize; residual-add + bn_stats per
            # 128-column chunk. po banks are retired by the copies; the
            # transpose tiles ping-pong through the po tags.
            ots = []
            for p in range(2):
                ot = osb.tile([DK + 1, 512], f32, tag=f"ot{p}", name="ot")
                nc.scalar.activation(out=ot, in_=po[p], func=AF.Relu)
                ots.append(ot)
            for j in range(4):
                qi = qb * 4 + j
                for p in range(2):
                    h = 2 * hp + p
                    pt = ps_attn.tile([128, DK + 1], f32, tag=f"po{(2 * j + p) % 2}", name="ps_t")
                    nc.tensor.transpose(
                        pt, ots[p][:, 128 * j : 128 * (j + 1)], id_sb[0 : DK + 1, 0 : DK + 1]
                    )
                    rc = smalls.tile([128, 1], f32, tag="rc", name="rc")
                    nc.vector.reciprocal(rc, pt[:, DK : DK + 1])
                    nc.scalar.activation(
                        out=om_sb[qi][:, DK * h : DK * (h + 1)],
                        in_=pt[:, 0:DK],
                        func=AF.Relu,
                        scale=rc,
                    )
                cols = slice(128 * hp, 128 * (hp + 1))
                nc.vector.tensor_add(
                    out=om_sb[qi][:, cols], in0=om_sb[qi][:, cols], in1=qr_sb[qi][:, cols]
                )
                nc.vector.bn_stats(out=st_sb[qi][:, hp, :], in_=om_sb[qi][:, cols])
                if hp == DT - 1:
                    layernorm(qi)

        # ---------------- layernorm finalize ----------------
        def layernorm(qi):
            mv = statp.tile([128, 2], f32, tag="mv", name="mv")
            nc.vector.bn_aggr(out=mv, in_=st_sb[qi])
            # rstd = (var*n/(n-1) + eps)^-0.5 via Ln+Exp (one ACT table set)
            lnv = statp.tile([128, 1], f32, tag="lnv", name="lnv")
            nc.scalar.activation(
                out=lnv, in_=mv[:, 1:2], func=AF.Ln, scale=float(D) / (D - 1), bias=eps_sb
            )
            rstd = statp.tile([128, 1], f32, tag="rstd", name="rstd")
            nc.scalar.activation(out=rstd, in_=lnv, func=AF.Exp, scale=-0.5)
            fo = fop.tile([128, D], bf16, tag=f"fo{qi % 2}", name="fo")
            nc.vector.tensor_scalar(
                out=fo,
                in0=om_sb[qi],
                scalar1=mv[:, 0:1],
                scalar2=rstd,
                op0=ALU.subtract,
                op1=ALU.mult,
            )
            # tail layernorms (qb1): gamma/beta on the then-idle VectorE;
            # inline ones (qb0) stay on GpSimd to keep VectorE on its exp cycle
            eng = nc.vector if qi >= 4 else nc.gpsimd
            eng.tensor_mul(out=fo, in0=fo, in1=gam_sb)
            eng.tensor_add(out=fo, in0=fo, in1=bet_sb)
            nc.sync.dma_start(out=out[128 * qi : 128 * (qi + 1), :], in_=fo)

        # ---------------- emission schedule ----------------
        # Pre-attention: only the units whose DMAs land first. K-projection
        # l-block lb is consumed by scores(kp=2lb), so the remaining blocks
        # ride as fillers far enough ahead (lb2/lb3 need kT's second half,
        # which lands latest).
        for which, lb in QK_UNITS[:3]:
            proj_qk_unit(0, which, lb)

        def filler_sched(hp, qb):
            sched = [[] for _ in range(NKP)]
            if hp == 0 and qb == 0:
                for kp in range(NKP):
                    sched[kp] = [
                        (lambda t=2 * kp: proj_v_unit(t)),
                        (lambda t=2 * kp + 1: proj_v_unit(t)),
                    ]
                sched[0].insert(0, lambda: proj_qk_unit(0, "k", 1))
                sched[3].insert(0, lambda: proj_qk_unit(0, "k", 2))
                sched[4].insert(0, lambda: proj_qk_unit(0, "k", 3))
            elif hp == 0 and qb == 1:
                for u, (which, lb) in enumerate(QK_UNITS):
                    kp = min(u + 1, NKP - 1)
                    sched[kp].append(lambda w=which, l=lb: proj_qk_unit(1, w, l))
            elif hp < DT - 1:
                halfu = QK_UNITS[:3] if qb == 0 else QK_UNITS[3:]
                for u, (which, lb) in enumerate(halfu):
                    kp = min(2 * u + 1, NKP - 1)
                    sched[kp].append(lambda i=hp + 1, w=which, l=lb: proj_qk_unit(i, w, l))
            return sched

        for hp in range(DT):
            for qb in range(2):
                attention(hp, qb, filler_sched(hp, qb))


def _build():
    global _COMPILED
    if _COMPILED is not None:
        return _COMPILED
    import concourse.bacc as bacc
    import concourse.tile as tile
    from concourse import mybir

    f32 = mybir.dt.float32
    bf16 = mybir.dt.bfloat16

    # Keep Exp/Ln in one ACT table set so a single table load serves all.
    if not getattr(bacc, "_act_tables_patched", False):
        _orig_get = bacc.get_activation_tables

        def _patched(arch):
            tables = _orig_get(arch)
            AF = mybir.ActivationFunctionType
            combined = "natural_log_exp_and_others"
            if combined in tables:
                for name, funcs in tables.items():
                    if name != combined:
                        funcs.discard(AF.Exp)
                        funcs.discard(AF.Ln)
            return tables

        bacc.get_activation_tables = _patched
        bacc._act_tables_patched = True

    nc = bacc.Bacc("TRN2", target_bir_lowering=False, debug=False, num_devices=N_CORES)
    aps = (
        nc.dram_tensor("qT", [D, LQ], bf16, kind="ExternalInput").ap(),
        nc.dram_tensor("kT", [D, LK], bf16, kind="ExternalInput").ap(),
        nc.dram_tensor("qres", [LQ, D], bf16, kind="ExternalInput").ap(),
        nc.dram_tensor("wqT", [D, D], bf16, kind="ExternalInput").ap(),
        nc.dram_tensor("wkT", [D, D], bf16, kind="ExternalInput").ap(),
        nc.dram_tensor("wvT", [D, D], bf16, kind="ExternalInput").ap(),
        nc.dram_tensor("bq8", [D], f32, kind="ExternalInput").ap(),
        nc.dram_tensor("bkv", [D], f32, kind="ExternalInput").ap(),
        nc.dram_tensor("bvb", [D], bf16, kind="ExternalInput").ap(),
        nc.dram_tensor("gam", [D], bf16, kind="ExternalInput").ap(),
        nc.dram_tensor("bet", [D], bf16, kind="ExternalInput").ap(),
        nc.dram_tensor("iden", [128, 128], f32, kind="ExternalInput").ap(),
        nc.dram_tensor("out", [LQ, D], bf16, kind="ExternalOutput").ap(),
    )
    with tile.TileContext(nc) as tc:
        _emit(tc, aps)
    nc.compile()
    _COMPILED = nc
    return nc


def _in_maps(inputs):
    bf = ml_dtypes.bfloat16
    q = np.asarray(inputs["query"], np.float32)
    k = np.asarray(inputs["key"], np.float32)
    sc = A16 / 8.0
    shared = {
        "wqT": np.ascontiguousarray((np.asarray(inputs["Wq"], np.float32) * sc).T).astype(bf),
        "wkT": np.ascontiguousarray(np.asarray(inputs["Wk"], np.float32).T).astype(bf),
        "wvT": np.ascontiguousarray(np.asarray(inputs["Wv"], np.float32).T).astype(bf),
        "bq8": np.asarray(inputs["bq"], np.float32) * sc,
        "bkv": np.asarray(inputs["bk"], np.float32),
        "bvb": np.asarray(inputs["bv"], np.float32).astype(bf),
        "gam": np.asarray(inputs["gamma"], np.float32).astype(bf),
        "bet": np.asarray(inputs["beta"], np.float32).astype(bf),
        "iden": np.eye(128, dtype=np.float32),
    }
    maps = []
    for c in range(N_CORES):
        b, hf = divmod(c, 2)
        qs = q[b, hf * LQ : (hf + 1) * LQ]
        maps.append(
            {
                "qT": np.ascontiguousarray(qs.T).astype(bf),
                "kT": np.ascontiguousarray(k[b].T).astype(bf),
                "qres": np.ascontiguousarray(qs).astype(bf),
                **shared,
            }
        )
    return maps


def _assemble(results):
    out = np.empty((B, L, D), np.float32)
    for c in range(N_CORES):
        b, hf = divmod(c, 2)
        out[b, hf * LQ : (hf + 1) * LQ] = results[c]["out"].astype(np.float32)
    return out


def kernel(**inputs) -> np.ndarray:
    from concourse.bass_utils import run_bass_kernel_spmd

    nc = _build()
    res = run_bass_kernel_spmd(nc, _in_maps(inputs), list(range(N_CORES)))
    return _assemble(res.results)


def _install_ntff_hook():
    """Make `antenv.axon_hooks` importable (the image's antenv lacks it)."""
    import importlib.util

    if "antenv.axon_hooks" in sys.modules:
        return
    spec = importlib.util.spec_from_file_location(
        "antenv.axon_hooks", "/opt/trn_rl_repo/antenv/axon_hooks.py"
    )
    mod = importlib.util.module_from_spec(spec)
    sys.modules["antenv.axon_hooks"] = mod
    spec.loader.exec_module(mod)


def run_traced(inputs, **trace_kwargs):
    """Like kernel() but with NTFF tracing; returns (out, BassKernelResults)."""
    from concourse.bass_utils import run_bass_kernel_spmd

    _install_ntff_hook()

    nc = _build()
    res = run_bass_kernel_spmd(
        nc, _in_maps(inputs), list(range(N_CORES)), trace=True, **trace_kwargs
    )
    return _assemble(res.results), res


# revision 27
# speedup vs baseline: 1.1885x; 1.0589x over previous
"""MultiHeadAttention (QKV proj + softmax attention + residual + LayerNorm)
for Trainium2, SPMD across 8 NeuronCores.

Sharding: data-parallel over (batch, query-L-half): core c handles batch c//2,
query rows [1024*(c%2), 1024*(c%2)+1024), all 12 heads, full 2048 keys.
No cross-core communication.

Structure (v10, ~371us vs 606us staged baseline):
- Paired-chunk input DMAs on the two HWDGE rings (sync/scalar), ordered by
  first use; gpsimd SWDGE only carries late-needed tensors.
- Q/K projection e-chunk 0 runs first; attention follows with the V
  projection and remaining Q/K projection chunks emitted as PE filler
  inside the attention loop.
- The attention inner loop is paced by the score-psum ping-pong cycle
  (scores -> exp -> scores, single-buffered due to the 8-bank PSUM budget).
  The two heads of a pair therefore run their exp on DIFFERENT engines
  (head0 exact exp on ScalarE; head1 Schraudolph bit-trick on VectorE:
  int8(A*s+B) bitcast to fp8e4m3), so the two cycles progress in parallel;
  attnV trails scores by one kp so nothing waits on a just-issued exp.
- attnV runs in fp8-e4m3 DoubleRow (contraction 256 = both k-chunks of a
  kp in one matmul). exp outputs are fp8 with a -4 shift (softmax is
  shift-invariant; keeps e^s inside e4m3 range, max score ~8.5).
- Out-path copies/normalize run on ScalarE (attn outputs are >=0, so Relu
  with per-partition scale=1/denominator is an exact normalize), keeping
  VectorE free for its exp share. Residual-add + bn_stats run inline per
  128-column chunk; the layernorm tail only aggregates + scales. Final
  output bf16 (restored to f32 on host).

Numerics: projections/scores in bf16 (fp32 accumulate), attnV fp8,
normalization + layernorm f32, residual/gamma/beta/output bf16. Scale
A16/8 (A16=2^7/ln2) is folded into Wq/bq on the host. End-to-end rel err
~6e-3 vs the 2e-2 gate (validated in numpy sim + on HW).
"""

import sys

sys.path.insert(0, "/opt/trn_rl_repo")

import numpy as np
import ml_dtypes

N_CORES = 8
B, L, D = 4, 2048, 768
H, DK = 12, 64
LQ = L // 2  # 1024 query rows per core
LK = L  # full keys per core
DT = D // 128  # 6 d-chunks
NKC = LK // 128  # 16 k-chunks
NKP = NKC // 2  # 8 kp iterations (2 k-chunks each)
VH = 80  # per-head stride in the fp8 V tile (16B-aligned)

A16 = 128.0 / float(np.log(2.0))  # folded score scale (2^7/ln2)
CSH = 4.0  # softmax shift: exp(s - CSH)
A8 = 8.0 / float(np.log(2.0))  # e4m3 Schraudolph scale
B8 = 56.0 - A8 * CSH - 0.47  # e4m3 exponent bias - shift - mid correction


def _use_dve_exp(hp, qb, kp, p):
    """Head p1's exp runs on VectorE (Schraudolph) so the two score-psum
    ping-pong cycles pace on different engines in parallel."""
    return p == 1


_COMPILED = None


def _emit(tc, aps):
    import contextlib

    import concourse.bass as bass
    from concourse import mybir

    nc = tc.nc
    f32 = mybir.dt.float32
    bf16 = mybir.dt.bfloat16
    fp8 = mybir.dt.float8e4
    i8 = mybir.dt.int8
    AF = mybir.ActivationFunctionType
    ALU = mybir.AluOpType
    PM = mybir.MatmulPerfMode

    qT, kT, qres, wqT, wkT, wvT, bq8, bkv, bvb, gam, bet, iden, out = aps

    ctx = contextlib.ExitStack()
    with ctx:
        const = ctx.enter_context(tc.tile_pool(name="const", bufs=1))
        persist = ctx.enter_context(tc.tile_pool(name="persist", bufs=1))
        # PSUM (8 banks of 2KB): s0,s1 [128,1024]f32 = 2 banks each; po0,po1
        # [65,512] 1 bank each (transpose tiles reuse them); pj0,pj1
        # (projection accumulators, ping-pong) 1 bank each.
        ps_score = ctx.enter_context(tc.tile_pool(name="ps_score", bufs=1, space="PSUM"))
        ps_attn = ctx.enter_context(tc.tile_pool(name="ps_attn", bufs=1, space="PSUM"))
        ps_proj = ctx.enter_context(tc.tile_pool(name="ps_proj", bufs=1, space="PSUM"))
        expp = ctx.enter_context(tc.tile_pool(name="expp", bufs=6))
        osb = ctx.enter_context(tc.tile_pool(name="osb", bufs=2))
        smalls = ctx.enter_context(tc.tile_pool(name="smalls", bufs=4))
        qrp = ctx.enter_context(tc.tile_pool(name="qrp", bufs=1))
        statp = ctx.enter_context(tc.tile_pool(name="statp", bufs=4))
        fop = ctx.enter_context(tc.tile_pool(name="fop", bufs=2))

        # ---------------- input tiles ----------------
        # Paired-chunk dma_starts (amortize per-transfer fixed cost) spread
        # round-robin over four issue queues; order = first use.
        _rings = [nc.sync, nc.scalar]
        _ring_rr = [0]

        def _dma(out_ap, in_ap):
            eng = _rings[_ring_rr[0] % len(_rings)]
            _ring_rr[0] += 1
            eng.dma_start(out=out_ap, in_=in_ap)

        def big_tile(name, ncols):
            t = const.tile([128, DT, ncols], bf16, tag=name, name=name)
            return t, [t[:, i, :] for i in range(DT)]

        wq_t, wq_sb = big_tile("wqc", D)
        qT_t, qT_sb = big_tile("qTc", LQ)
        wk_t, wk_sb = big_tile("wkc", D)
        kT_t, kT_sb = big_tile("kTc", LK)
        wv_t, wv_sb = big_tile("wvc", D)

        def pair_dmas(t, dram, ncols, c0=0, c1=None):
            for i in range(DT // 2):
                src = bass.AP(
                    tensor=dram.tensor,
                    offset=dram.offset + 256 * i * ncols + c0,
                    ap=[[ncols, 128], [128 * ncols, 2], [1, (c1 or ncols) - c0]],
                )
                _dma(t[:, 2 * i : 2 * i + 2, c0 : c1 or ncols], src)

        pair_dmas(wq_t, wqT, D)
        pair_dmas(qT_t, qT, LQ)
        pair_dmas(wk_t, wkT, D)
        pair_dmas(kT_t, kT, LK, 0, 1024)
        pair_dmas(wv_t, wvT, D)
        pair_dmas(kT_t, kT, LK, 1024, 2048)

        def load_bias(name, dram, eng):
            t = const.tile([128, DT, 1], f32, tag=name, name=name)
            src = bass.AP(
                tensor=dram.tensor, offset=dram.offset, ap=[[1, 128], [128, DT], [0, 1]]
            )
            eng.dma_start(out=t, in_=src)
            return [t[:, i, :] for i in range(DT)]

        bq_sb = load_bias("bq", bq8, nc.sync)
        bk_sb = load_bias("bk", bkv, nc.scalar)
        bv_sb = const.tile([1, D], bf16, tag="bv", name="bv_sb")
        nc.sync.dma_start(out=bv_sb, in_=bvb[:])
        id_sb = const.tile([65, 65], f32, tag="iden", name="id_sb")
        nc.gpsimd.dma_start(out=id_sb, in_=iden[0:65, 0:65])
        gam_sb = const.tile([128, D], bf16, tag="gam", name="gam_sb")
        nc.gpsimd.dma_start(
            out=gam_sb,
            in_=bass.AP(tensor=gam.tensor, offset=gam.offset, ap=[[0, 128]] + list(gam.ap)),
        )
        bet_sb = const.tile([128, D], bf16, tag="bet", name="bet_sb")
        nc.gpsimd.dma_start(
            out=bet_sb,
            in_=bass.AP(tensor=bet.tensor, offset=bet.offset, ap=[[0, 128]] + list(bet.ap)),
        )
        ones_sb = const.tile([1, 128], bf16, tag="ones", name="ones_sb")
        nc.vector.memset(ones_sb, 1.0)
        eps_sb = const.tile([128, 1], f32, tag="eps", name="eps_sb")
        nc.vector.memset(eps_sb, 1e-5)
        csh_sb = const.tile([128, 1], f32, tag="csh", name="csh_sb")
        nc.vector.memset(csh_sb, -CSH)

        # residual tiles (first needed ~45us in)
        qr_sb = [qrp.tile([128, D], bf16, tag=f"qr{qi}", name=f"qr{qi}") for qi in range(LQ // 128)]
        for qi in range(LQ // 128):
            (nc.gpsimd if qi % 2 else nc.sync).dma_start(
                out=qr_sb[qi], in_=qres[128 * qi : 128 * (qi + 1), :]
            )

        # persistent intermediates
        pq_sb = [persist.tile([128, LQ], bf16, tag=f"pq{i}", name=f"pq_sb{i}") for i in range(DT)]
        pk_sb = [persist.tile([128, LK], bf16, tag=f"pk{i}", name=f"pk_sb{i}") for i in range(DT)]
        # V in fp8, kc pairs interleaved for DoubleRow: [l-part, pair, head, VH]
        v_sb = [
            persist.tile([128, 2, H, VH], fp8, tag=f"v{t}", name=f"v_sb{t}")
            for t in range(NKP)
        ]
        om_sb = [persist.tile([128, D], f32, tag=f"om{q}", name=f"om_sb{q}") for q in range(LQ // 128)]
        st_sb = [persist.tile([128, DT, 6], f32, tag=f"st{q}", name=f"st_sb{q}") for q in range(LQ // 128)]

        # ---------------- projection units (emitted as PE filler) ----------
        _pj_rr = [0]

        def _pj_tile():
            ps = ps_proj.tile([128, 512], f32, tag=f"pj{_pj_rr[0] % 2}", name="ps_p")
            _pj_rr[0] += 1
            return ps

        def proj_qk_unit(i, which, lb):
            src_tiles, w_tiles, b_tiles, dst = (
                (qT_sb, wq_sb, bq_sb, pq_sb) if which == "q" else (kT_sb, wk_sb, bk_sb, pk_sb)
            )
            ps = _pj_tile()
            for kc in range(DT):
                nc.tensor.matmul(
                    ps,
                    lhsT=w_tiles[kc][:, 128 * i : 128 * (i + 1)],
                    rhs=src_tiles[kc][:, 512 * lb : 512 * (lb + 1)],
                    start=kc == 0,
                    stop=kc == DT - 1,
                )
            nc.scalar.activation(
                out=dst[i][:, 512 * lb : 512 * (lb + 1)],
                in_=ps,
                func=AF.Relu,
                bias=b_tiles[i],
            )

        QK_UNITS = [("q", 0), ("q", 1), ("k", 0), ("k", 1), ("k", 2), ("k", 3)]

        def proj_v_unit(t):
            kp2, sl = divmod(t, 2)
            if sl == 0:
                nc.vector.memset(v_sb[kp2][:, :, :, DK : DK + 1], 1.0)
            for ei, (e0, ew) in enumerate(((0, 512), (512, 256))):
                ps = _pj_tile()
                for kc in range(DT):
                    nc.tensor.matmul(
                        ps[:, 0:ew],
                        lhsT=kT_sb[kc][:, 128 * t : 128 * (t + 1)],
                        rhs=wv_sb[kc][:, e0 : e0 + ew],
                        start=kc == 0,
                        stop=False,
                    )
                nc.tensor.matmul(
                    ps[:, 0:ew],
                    lhsT=ones_sb[:, 0:128],
                    rhs=bv_sb[:, e0 : e0 + ew],
                    start=False,
                    stop=True,
                )
                nc.vector.tensor_scalar(
                    out=v_sb[kp2][:, sl, e0 // DK : (e0 + ew) // DK, 0:DK],
                    in0=ps[:, 0:ew].rearrange("p (h x) -> p h x", x=DK),
                    scalar1=0.0,
                    scalar2=None,
                    op0=ALU.max,
                )

        # ---------------- attention ----------------
        def attention(hp, qb, filler):
            po = [
                ps_attn.tile([DK + 1, 512], f32, tag=f"po{p}", name=f"ps_o{p}")
                for p in range(2)
            ]

            def attn_v1(kp, p, e_t):
                # one fp8 DoubleRow matmul: contraction over both k-chunks
                # of kp (pair dim), accumulating into po[p].
                h = 2 * hp + p
                nc.tensor.matmul(
                    po[p],
                    lhsT=v_sb[kp][:, :, h, 0 : DK + 1],
                    rhs=e_t.rearrange("x (two q) -> x two q", two=2),
                    start=kp == 0,
                    stop=kp == NKP - 1,
                    perf_mode=PM.DoubleRow,
                )

            def scores(kp, p, ps):
                # scores^T [k-part, q-free]; head 2hp+p row-tiled (K=64)
                for half in range(2):
                    kc = 2 * kp + half
                    nc.tensor.matmul(
                        ps[:, 512 * half : 512 * (half + 1)],
                        lhsT=pk_sb[hp][64 * p : 64 * (p + 1), 128 * kc : 128 * (kc + 1)],
                        rhs=pq_sb[hp][64 * p : 64 * (p + 1), 512 * qb : 512 * (qb + 1)],
                        start=True,
                        stop=True,
                        tile_position=(64 * p, 0),
                    )

            def exp(kp, p, ps):
                e_t = expp.tile([128, 1024], fp8, tag="exp", bufs=8, name="e_t")
                if _use_dve_exp(hp, qb, kp, p):
                    nc.vector.tensor_scalar(
                        out=e_t.bitcast(i8),
                        in0=ps,
                        scalar1=1.0 / 16.0,
                        scalar2=B8,
                        op0=ALU.mult,
                        op1=ALU.add,
                    )
                else:
                    nc.scalar.activation(
                        out=e_t, in_=ps, func=AF.Exp, scale=1.0 / A16, bias=csh_sb
                    )
                return e_t

            # Software pipeline: each head's score matmuls sit ~one block
            # after the exp that frees their psum tile, so nothing stalls;
            # attnV for kp-1 consumes the previous block's exp output.
            prev_e = [None, None]
            for kp in range(NKP):
                pspair = [
                    ps_score.tile([128, 1024], f32, tag=f"s{p}", name=f"ps_s{p}")
                    for p in range(2)
                ]
                scores(kp, 0, pspair[0])
                e0 = exp(kp, 0, pspair[0])
                if kp > 0:
                    attn_v1(kp - 1, 0, prev_e[0])
                scores(kp, 1, pspair[1])
                e1 = exp(kp, 1, pspair[1])
                if kp > 0:
                    attn_v1(kp - 1, 1, prev_e[1])
                for f in filler[kp]:
                    f()
                prev_e = [e0, e1]
            attn_v1(NKP - 1, 0, prev_e[0])
            attn_v1(NKP - 1, 1, prev_e[1])

            # evacuate + transpose + normalize; residual-add + bn_stats per
            # 128-column chunk. po banks are retired by the copies; the
            # transpose tiles ping-pong through the po tags.
            ots = []
            for p in range(2):
                ot = osb.tile([DK + 1, 512], f32, tag=f"ot{p}", name="ot")
                nc.scalar.activation(out=ot, in_=po[p], func=AF.Relu)
                ots.append(ot)
            for j in range(4):
                qi = qb * 4 + j
                for p in range(2):
                    h = 2 * hp + p
                    pt = ps_attn.tile([128, DK + 1], f32, tag=f"po{(2 * j + p) % 2}", name="ps_t")
                    nc.tensor.transpose(
                        pt, ots[p][:, 128 * j : 128 * (j + 1)], id_sb[0 : DK + 1, 0 : DK + 1]
                    )
                    rc = smalls.tile([128, 1], f32, tag="rc", name="rc")
                    nc.vector.reciprocal(rc, pt[:, DK : DK + 1])
                    nc.scalar.activation(
                        out=om_sb[qi][:, DK * h : DK * (h + 1)],
                        in_=pt[:, 0:DK],
                        func=AF.Relu,
                        scale=rc,
                    )
                cols = slice(128 * hp, 128 * (hp + 1))
                nc.vector.tensor_add(
                    out=om_sb[qi][:, cols], in0=om_sb[qi][:, cols], in1=qr_sb[qi][:, cols]
                )
                nc.vector.bn_stats(out=st_sb[qi][:, hp, :], in_=om_sb[qi][:, cols])
                if hp == DT - 1:
                    layernorm(qi)

        # ---------------- layernorm finalize ----------------
        def layernorm(qi):
            mv = statp.tile([128, 2], f32, tag="mv", name="mv")
            nc.vector.bn_aggr(out=mv, in_=st_sb[qi])
            # rstd = (var*n/(n-1) + eps)^-0.5 via Ln+Exp (one ACT table set)
            lnv = statp.tile([128, 1], f32, tag="lnv", name="lnv")
            nc.scalar.activation(
                out=lnv, in_=mv[:, 1:2], func=AF.Ln, scale=float(D) / (D - 1), bias=eps_sb
            )
            rstd = statp.tile([128, 1], f32, tag="rstd", name="rstd")
            nc.scalar.activation(out=rstd, in_=lnv, func=AF.Exp, scale=-0.5)
            fo = fop.tile([128, D], bf16, tag=f"fo{qi % 2}", name="fo")
            nc.vector.tensor_scalar(
                out=fo,
                in0=om_sb[qi],
                scalar1=mv[:, 0:1],
                scalar2=rstd,
                op0=ALU.subtract,
                op1=ALU.mult,
            )
            # tail layernorms (qb1): gamma/beta on the then-idle VectorE;
            # inline ones (qb0) stay on GpSimd to keep VectorE on its exp cycle
            eng = nc.vector if qi >= 4 else nc.gpsimd
            eng.tensor_mul(out=fo, in0=fo, in1=gam_sb)
            eng.tensor_add(out=fo, in0=fo, in1=bet_sb)
            nc.sync.dma_start(out=out[128 * qi : 128 * (qi + 1), :], in_=fo)

        # ---------------- emission schedule ----------------
        # Pre-attention: only the units whose DMAs land first. K-projection
        # l-block lb is consumed by scores(kp=2lb), so the remaining blocks
        # ride as fillers far enough ahead (lb2/lb3 need kT's second half,
        # which lands latest).
        for which, lb in QK_UNITS[:3]:
            proj_qk_unit(0, which, lb)

        def filler_sched(hp, qb):
            sched = [[] for _ in range(NKP)]
            if hp == 0 and qb == 0:
                for kp in range(NKP):
                    sched[kp] = [
                        (lambda t=2 * kp: proj_v_unit(t)),
                        (lambda t=2 * kp + 1: proj_v_unit(t)),
                    ]
                sched[0].insert(0, lambda: proj_qk_unit(0, "k", 1))
                sched[3].insert(0, lambda: proj_qk_unit(0, "k", 2))
                sched[4].insert(0, lambda: proj_qk_unit(0, "k", 3))
            elif hp == 0 and qb == 1:
                for u, (which, lb) in enumerate(QK_UNITS):
                    kp = min(u + 1, NKP - 1)
                    sched[kp].append(lambda w=which, l=lb: proj_qk_unit(1, w, l))
            elif hp < DT - 1:
                halfu = QK_UNITS[:3] if qb == 0 else QK_UNITS[3:]
                for u, (which, lb) in enumerate(halfu):
                    kp = min(2 * u + 1, NKP - 1)
                    sched[kp].append(lambda i=hp + 1, w=which, l=lb: proj_qk_unit(i, w, l))
            return sched

        for hp in range(DT):
            for qb in range(2):
                attention(hp, qb, filler_sched(hp, qb))


def _build():
    global _COMPILED
    if _COMPILED is not None:
        return _COMPILED
    import concourse.bacc as bacc
    import concourse.tile as tile
    from concourse import mybir

    f32 = mybir.dt.float32
    bf16 = mybir.dt.bfloat16

    # Keep Exp/Ln in one ACT table set so a single table load serves all.
    if not getattr(bacc, "_act_tables_patched", False):
        _orig_get = bacc.get_activation_tables

        def _patched(arch):
            tables = _orig_get(arch)
            AF = mybir.ActivationFunctionType
            combined = "natural_log_exp_and_others"
            if combined in tables:
                for name, funcs in tables.items():
                    if name != combined:
                        funcs.discard(AF.Exp)
                        funcs.discard(AF.Ln)
            return tables

        bacc.get_activation_tables = _patched
        bacc._act_tables_patched = True

    nc = bacc.Bacc("TRN2", target_bir_lowering=False, debug=False, num_devices=N_CORES)
    aps = (
        nc.dram_tensor("qT", [D, LQ], bf16, kind="ExternalInput").ap(),
        nc.dram_tensor("kT", [D, LK], bf16, kind="ExternalInput").ap(),
        nc.dram_tensor("qres", [LQ, D], bf16, kind="ExternalInput").ap(),
        nc.dram_tensor("wqT", [D, D], bf16, kind="ExternalInput").ap(),
        nc.dram_tensor("wkT", [D, D], bf16, kind="ExternalInput").ap(),
        nc.dram_tensor("wvT", [D, D], bf16, kind="ExternalInput").ap(),
        nc.dram_tensor("bq8", [D], f32, kind="ExternalInput").ap(),
        nc.dram_tensor("bkv", [D], f32, kind="ExternalInput").ap(),
        nc.dram_tensor("bvb", [D], bf16, kind="ExternalInput").ap(),
        nc.dram_tensor("gam", [D], bf16, kind="ExternalInput").ap(),
        nc.dram_tensor("bet", [D], bf16, kind="ExternalInput").ap(),
        nc.dram_tensor("iden", [128, 128], f32, kind="ExternalInput").ap(),
        nc.dram_tensor("out", [LQ, D], bf16, kind="ExternalOutput").ap(),
    )
    with tile.TileContext(nc) as tc:
        _emit(tc, aps)
    nc.compile()
    _COMPILED = nc
    return nc


def _in_maps(inputs):
    bf = ml_dtypes.bfloat16
    q = np.asarray(inputs["query"], np.float32)
    k = np.asarray(inputs["key"], np.float32)
    sc = A16 / 8.0
    shared = {
        "wqT": np.ascontiguousarray((np.asarray(inputs["Wq"], np.float32) * sc).T).astype(bf),
        "wkT": np.ascontiguousarray(np.asarray(inputs["Wk"], np.float32).T).astype(bf),
        "wvT": np.ascontiguousarray(np.asarray(inputs["Wv"], np.float32).T).astype(bf),
        "bq8": np.asarray(inputs["bq"], np.float32) * sc,
        "bkv": np.asarray(inputs["bk"], np.float32),
        "bvb": np.asarray(inputs["bv"], np.float32).astype(bf),
        "gam": np.asarray(inputs["gamma"], np.float32).astype(bf),
        "bet": np.asarray(inputs["beta"], np.float32).astype(bf),
        "iden": np.eye(128, dtype=np.float32),
    }
    maps = []
    for c in range(N_CORES):
        b, hf = divmod(c, 2)
        qs = q[b, hf * LQ : (hf + 1) * LQ]
        maps.append(
            {
                "qT": np.ascontiguousarray(qs.T).astype(bf),
                "kT": np.ascontiguousarray(k[b].T).astype(bf),
                "qres": np.ascontiguousarray(qs).astype(bf),
                **shared,
            }
        )
    return maps


def _assemble(results):
    out = np.empty((B, L, D), np.float32)
    for c in range(N_CORES):
        b, hf = divmod(c, 2)
        out[b, hf * LQ : (hf + 1) * LQ] = results[c]["out"].astype(np.float32)
    return out


def kernel(**inputs) -> np.ndarray:
    from concourse.bass_utils import run_bass_kernel_spmd

    nc = _build()
    res = run_bass_kernel_spmd(nc, _in_maps(inputs), list(range(N_CORES)))
    return _assemble(res.results)


def _install_ntff_hook():
    """Make `antenv.axon_hooks` importable (the image's antenv lacks it)."""
    import importlib.util

    if "antenv.axon_hooks" in sys.modules:
        return
    spec = importlib.util.spec_from_file_location(
        "antenv.axon_hooks", "/opt/trn_rl_repo/antenv/axon_hooks.py"
    )
    mod = importlib.util.module_from_spec(spec)
    sys.modules["antenv.axon_hooks"] = mod
    spec.loader.exec_module(mod)


def run_traced(inputs, **trace_kwargs):
    """Like kernel() but with NTFF tracing; returns (out, BassKernelResults)."""
    from concourse.bass_utils import run_bass_kernel_spmd

    _install_ntff_hook()

    nc = _build()
    res = run_bass_kernel_spmd(
        nc, _in_maps(inputs), list(range(N_CORES)), trace=True, **trace_kwargs
    )
    return _assemble(res.results), res
